# revision 1
# baseline (speedup 1.0000x reference)
"""Multi-head self-attention Trainium2 kernel (8 NeuronCores).

Sharding: 8 cores = 4 batches x 2 head-groups (8 heads each).
Core c handles batch b=c//2, heads [g*8, (g+1)*8) where g=c%2.
Each core computes a partial output (its heads' contribution to the
output projection); the host sums the two partials per batch and adds bo.

All matmuls run in float32r (fp32 data, ~1 cycle/row vs 4 for fp32,
~1.5e-4 matmul rel err). fp32r matmuls require output base partition 0.

Per-core dataflow:
  xT [1024, 2048] (= x[b].T), wq/wk/wv [1024, 512], wo [512, 1024]
  A1: QT[p]/KT[p] = w_p.T @ x.T  [128, 2048] per head-pair p (2 heads x 64
      dims on partitions). PSUM accum over 8 k-tiles.
  A2: VS[jt] = [x_jt @ wv | ones] per 128-token tile: [128, 8*65] with a
      ones column per head (the ones column makes the PV matmul emit the
      softmax normalizer as row 64 of the context tile).
  B:  per (pair p, 512-query block qb):
        ST[j-tile, i] = KT_h-slice.T x QT_h  (K=64, head pair row-packed)
        PT = exp(0.125 * ST)                 (ScalarE, 1536/1024-elem groups)
        ct_par[c(65), i] += VS[jt]_h.T @ PT  (row 64 accumulates sum(exp))
        normalize: recip(row64) -> K=1 matmul broadcast -> DVE mult
        -> cth[h] [64, 512] per head (fp32r)
  C:  per qb: out[tokens, :] = sum_h cth[h].T-slice @ wo_h  (K=64 accum)
"""

import numpy as np

import concourse.bass as bass
import concourse.tile as tile
from concourse import bacc, mybir
from contextlib import ExitStack

P = 128
D = 1024
HD = 512  # head dims per core (8 heads x 64)
NPAIR = 4
NH = 8
F32 = mybir.dt.float32
FR = mybir.dt.float32r


def _st_groups(n_slices):
    """Split n_slices exp slices into alternating groups of 3 and 2."""
    groups = []
    want = 3
    rem = n_slices
    while rem > 0:
        g = min(want, rem)
        groups.append(g)
        rem -= g
        want = 2 if want == 3 else 3
    return groups


def build_nc(S=2048):
    NKT = D // P          # 8 k-tiles over model dim
    NJT = S // P          # key tiles
    MSEG = 512
    NMSEG = S // MSEG
    QB = 512
    NQB = S // QB

    nc = bacc.Bacc("TRN2", target_bir_lowering=False, debug=False)
    xT = nc.dram_tensor("xT", [D, S], FR, kind="ExternalInput").ap()
    wq = nc.dram_tensor("wq", [D, HD], FR, kind="ExternalInput").ap()
    wk = nc.dram_tensor("wk", [D, HD], FR, kind="ExternalInput").ap()
    wv = nc.dram_tensor("wv", [D, HD], FR, kind="ExternalInput").ap()
    wo = nc.dram_tensor("wo", [HD, D], FR, kind="ExternalInput").ap()
    out = nc.dram_tensor("out", [S, D], F32, kind="ExternalOutput").ap()

    with tile.TileContext(nc) as tc:
        with ExitStack() as persist:
            const_pool = persist.enter_context(tc.tile_pool(name="const", bufs=1))
            data_pool = persist.enter_context(tc.tile_pool(name="data", bufs=1))

            ones_f32 = const_pool.tile([P, 64], F32, tag="ones32", name="ones_f32")
            nc.vector.memset(ones_f32[:], 1.0)
            ones = const_pool.tile([P, 64], FR, tag="ones", name="ones")
            nc.vector.tensor_copy(ones[:], ones_f32[:])
            ones8_f32 = const_pool.tile([P, NH], F32, tag="ones8", name="ones8_f32")
            nc.vector.memset(ones8_f32[:], 1.0)

            QT = [data_pool.tile([P, S], FR, tag=f"qt{p}", name=f"qt{p}")
                  for p in range(NPAIR)]
            KT = [data_pool.tile([P, S], FR, tag=f"kt{p}", name=f"kt{p}")
                  for p in range(NPAIR)]
            # [128 tokens, 8 heads x (64 dims + ones col)]
            VS = [data_pool.tile([P, NH * 65], FR, tag=f"vs{j}", name=f"vs{j}")
                  for j in range(NJT)]

            # ---------------- Phase A: projections ----------------
            with ExitStack() as es_a:
                w_pool = es_a.enter_context(tc.tile_pool(name="wpool", bufs=1))
                chunk_pool = es_a.enter_context(tc.tile_pool(name="chunks", bufs=6))

                wq_t = w_pool.tile([P, NKT, HD], FR, tag="wq", name="wq_t")
                nc.sync.dma_start(wq_t[:], wq.rearrange("(kt p) n -> p kt n", p=P))
                wk_t = w_pool.tile([P, NKT, HD], FR, tag="wk", name="wk_t")
                nc.sync.dma_start(wk_t[:], wk.rearrange("(kt p) n -> p kt n", p=P))
                wv_t = w_pool.tile([P, NKT, HD], FR, tag="wv", name="wv_t")
                nc.sync.dma_start(wv_t[:], wv.rearrange("(kt p) n -> p kt n", p=P))

                # --- A1: QT / KT (8 PSUM accumulators: (q|k) x 4 pairs) ---
                with tc.tile_pool(name="qkps", bufs=8, space="PSUM") as qk_pool:
                    for mseg in range(NMSEG):
                        accs = [qk_pool.tile([P, MSEG], F32, tag="qk", name="qkacc")
                                for _ in range(8)]
                        for kt in range(NKT):
                            xc = chunk_pool.tile([P, MSEG], FR, tag="xc", name="xc")
                            nc.sync.dma_start(
                                xc[:],
                                xT[kt * P:(kt + 1) * P, mseg * MSEG:(mseg + 1) * MSEG])
                            for p in range(NPAIR):
                                for ti, wt in ((0, wq_t), (1, wk_t)):
                                    nc.tensor.matmul(
                                        accs[p * 2 + ti][:],
                                        lhsT=wt[:, kt, p * P:(p + 1) * P],
                                        rhs=xc[:],
                                        start=(kt == 0), stop=(kt == NKT - 1))
                        for p in range(NPAIR):
                            nc.vector.tensor_copy(
                                QT[p][:, mseg * MSEG:(mseg + 1) * MSEG], accs[p * 2][:])
                            nc.vector.tensor_copy(
                                KT[p][:, mseg * MSEG:(mseg + 1) * MSEG], accs[p * 2 + 1][:])

                # --- A2: V (natural layout, 4 j-tiles per mseg) ---
                with tc.tile_pool(name="vps", bufs=8, space="PSUM") as v_pool:
                    for mseg in range(NMSEG):
                        vaccs = [v_pool.tile([P, HD], F32, tag="v", name="vacc")
                                 for _ in range(4)]
                        for kt in range(NKT):
                            xc = chunk_pool.tile([P, MSEG], FR, tag="xc", name="xc")
                            nc.sync.dma_start(
                                xc[:],
                                xT[kt * P:(kt + 1) * P, mseg * MSEG:(mseg + 1) * MSEG])
                            for i in range(4):
                                nc.tensor.matmul(
                                    vaccs[i][:],
                                    lhsT=xc[:, i * P:(i + 1) * P],
                                    rhs=wv_t[:, kt, :],
                                    start=(kt == 0), stop=(kt == NKT - 1))
                        for i in range(4):
                            vsv = VS[mseg * 4 + i].rearrange("p (h c) -> p h c", c=65)
                            nc.vector.tensor_copy(vsv[:, :, 0:64], vaccs[i][:])
                            nc.vector.tensor_copy(vsv[:, :, 64], ones8_f32[:])

            # ---------------- Phases B + C: attention + projection ----------------
            with ExitStack() as es_b:
                cth_pool = es_b.enter_context(tc.tile_pool(name="cthpool", bufs=2))
                wo_pool = es_b.enter_context(tc.tile_pool(name="wopool", bufs=1))
                pt_pool = es_b.enter_context(tc.tile_pool(name="ptpool", bufs=2))
                rc_pool = es_b.enter_context(tc.tile_pool(name="rcpool", bufs=2))
                po_pool = es_b.enter_context(tc.tile_pool(name="popool", bufs=3))
                st_ps = es_b.enter_context(tc.tile_pool(name="stps", bufs=1, space="PSUM"))
                ct_ps = es_b.enter_context(tc.tile_pool(name="ctps", bufs=1, space="PSUM"))
                pj_ps = es_b.enter_context(tc.tile_pool(name="pjps", bufs=1, space="PSUM"))

                wo_h = []
                for h in range(NH):
                    t = wo_pool.tile([64, D], FR, tag=f"wo{h}", name=f"wo{h}")
                    nc.sync.dma_start(t[:], wo[h * 64:(h + 1) * 64, :])
                    wo_h.append(t)

                groups = _st_groups(2 * NJT)

                for qb in range(NQB):
                    cth = [None] * NH
                    for p in range(NPAIR):
                        cts = [ct_ps.tile([65, QB], F32, tag="cte", name="cte"),
                               ct_ps.tile([65, QB], F32, tag="cto", name="cto")]
                        s0 = 0
                        for gl in groups:
                            tag = "stA" if gl == 3 else "stB"
                            stg = st_ps.tile([P, gl * 512], F32, tag=tag, name="stg")
                            for l in range(gl):
                                s = s0 + l
                                jt, par = divmod(s, 2)
                                nc.tensor.matmul(
                                    stg[:, l * 512:(l + 1) * 512],
                                    lhsT=KT[p][par * 64:(par + 1) * 64,
                                               jt * P:(jt + 1) * P],
                                    rhs=QT[p][par * 64:(par + 1) * 64,
                                              qb * QB:(qb + 1) * QB],
                                    start=True, stop=True)
                            ptg = pt_pool.tile([P, gl * 512], FR, tag=tag, name="ptg")
                            nc.scalar.activation(
                                ptg[:], stg[:],
                                mybir.ActivationFunctionType.Exp, scale=0.125)
                            for l in range(gl):
                                s = s0 + l
                                jt, par = divmod(s, 2)
                                h = 2 * p + par
                                nc.tensor.matmul(
                                    cts[par][:],
                                    lhsT=VS[jt][:, h * 65:(h + 1) * 65],
                                    rhs=ptg[:, l * 512:(l + 1) * 512],
                                    start=(jt == 0), stop=(jt == NJT - 1))
                            s0 += gl

                        # normalize: cth[h] = ct[0:64] * (1/ct[64]) broadcast
                        for par in range(2):
                            h = 2 * p + par
                            ct = cts[par]
                            rc = rc_pool.tile([65, QB], FR, tag="rc", name="rc")
                            with nc.allow_low_precision(reason="softmax recip"):
                                nc.vector.reciprocal(rc[64:65, :], ct[64:65, :])
                            bc_ps = pj_ps.tile([P, QB], F32, tag="pj", name="bc_ps")
                            nc.tensor.matmul(bc_ps[0:64, :], lhsT=ones[64:65, :],
                                             rhs=rc[64:65, :], start=True, stop=True)
                            bc_sb = rc_pool.tile([64, QB], F32, tag="bc", name="bc_sb")
                            nc.vector.tensor_copy(bc_sb[:], bc_ps[0:64, :])
                            t = cth_pool.tile([64, QB], FR, tag=f"cth{h}",
                                              name=f"cth{h}")
                            nc.vector.tensor_tensor(
                                t[:], ct[0:64, :], bc_sb[:], mybir.AluOpType.mult)
                            cth[h] = t

                    # --- Phase C: project this query block's tokens ---
                    for mtl in range(4):
                        mt = qb * 4 + mtl
                        for half in range(2):
                            po = pj_ps.tile([P, 512], F32, tag="pj", name="po")
                            for h in range(NH):
                                nc.tensor.matmul(
                                    po[:],
                                    lhsT=cth[h][:, mtl * P:(mtl + 1) * P],
                                    rhs=wo_h[h][:, half * 512:(half + 1) * 512],
                                    start=(h == 0), stop=(h == NH - 1))
                            po_sb = po_pool.tile([P, 512], F32, tag="posb", name="po_sb")
                            nc.vector.tensor_copy(po_sb[:], po[:])
                            nc.sync.dma_start(
                                out[mt * P:(mt + 1) * P, half * 512:(half + 1) * 512],
                                po_sb[:])
    nc.compile()
    return nc


_NC_CACHE = {}


def _get_nc(S=2048):
    if S not in _NC_CACHE:
        _NC_CACHE[S] = build_nc(S)
    return _NC_CACHE[S]


def kernel(x, Wq, Wk, Wv, Wo, bo):
    from concourse.bass_utils import run_bass_kernel_spmd

    x = np.asarray(x, dtype=np.float32)
    Wq = np.asarray(Wq, dtype=np.float32)
    Wk = np.asarray(Wk, dtype=np.float32)
    Wv = np.asarray(Wv, dtype=np.float32)
    Wo = np.asarray(Wo, dtype=np.float32)
    bo = np.asarray(bo, dtype=np.float32)

    bs, S, d = x.shape
    nc = _get_nc(S)

    in_maps = []
    for c in range(8):
        b, g = divmod(c, 2)
        cols = slice(g * HD, (g + 1) * HD)
        in_maps.append({
            "xT": np.ascontiguousarray(x[b].T),
            "wq": np.ascontiguousarray(Wq[:, cols]),
            "wk": np.ascontiguousarray(Wk[:, cols]),
            "wv": np.ascontiguousarray(Wv[:, cols]),
            "wo": np.ascontiguousarray(Wo[cols, :]),
        })

    res = run_bass_kernel_spmd(nc, in_maps, core_ids=list(range(8)))
    outp = np.empty((bs, S, d), dtype=np.float32)
    for b in range(bs):
        outp[b] = res.results[2 * b]["out"] + res.results[2 * b + 1]["out"] + bo
    return outp



# revision 4
# speedup vs baseline: 1.7825x; 1.7825x over previous
"""Multi-head self-attention Trainium2 kernel (8 NeuronCores).

Sharding: 8 cores = 4 batches x 2 head-groups (8 heads each).
Core c handles batch b=c//2, heads [g*8, (g+1)*8) where g=c%2.
Each core computes a partial output (its heads' contribution to the
output projection); the host sums the two partials per batch and adds bo.

All matmuls run in float32r (fp32 data, ~1 cycle/row vs 4 for fp32,
~1.5e-4 matmul rel err). fp32r matmuls require output base partition 0.

Per-core dataflow (v2 — pipelined for ACT saturation + PE density):
  xT [1024, 2048] (= x[b].T), wq/wk/wv [1024, 512], wo [512, 1024]
  A (fused): per 512-token mseg, load all 8 x k-chunks ONCE into SBUF,
     then A1: QT[p]/KT[p] = w_p.T @ x.T (8 PSUM accs over k-tiles) and
     A2: VS[jt] = [x_jt @ wv | ones] reusing the same chunks.
  B: per (pair p, 512-query block qb), per key-tile jt (16):
       2 row-packed score MMs (par0 rows 0-63, par1 rows 64-127, K=64)
         -> stg [128, 1024] (2 PSUM banks, double-buffered by jt parity)
       exp via ScalarE on the full [128, 1024] group -> ptg (SBUF, f32r)
       2 PV MMs accumulate ct[par] [65, 512] over jt (ones col => row 64
         collects sum(exp) = softmax normalizer)
     normalize: recip_approx_fast on the two sum rows, GPSIMD
       partition_broadcast, DVE mult -> cth pair tile [128, 512]
       (two heads stacked on partitions for pair-packed C).
  C: out[tokens] = sum_p cth_p.T-slice @ wo_p (K=128, 4-pair PSUM accum);
     emitted software-pipelined inside the NEXT qb's pair loop so the
     PE work hides under the ACT-bound B phase.
"""

import numpy as np

import concourse.bass as bass
import concourse.tile as tile
from concourse import bacc, mybir
from contextlib import ExitStack

P = 128
D = 1024
HD = 512  # head dims per core (8 heads x 64)
NPAIR = 4
NH = 8
F32 = mybir.dt.float32
FR = mybir.dt.float32r


def build_nc(S=2048):
    NKT = D // P          # 8 k-tiles over model dim
    NJT = S // P          # 16 key tiles
    MSEG = 512
    NMSEG = S // MSEG
    QB = 512
    NQB = S // QB

    nc = bacc.Bacc("TRN2", target_bir_lowering=False, debug=False)
    xT = nc.dram_tensor("xT", [D, S], FR, kind="ExternalInput").ap()
    wq = nc.dram_tensor("wq", [D, HD], FR, kind="ExternalInput").ap()
    wk = nc.dram_tensor("wk", [D, HD], FR, kind="ExternalInput").ap()
    wv = nc.dram_tensor("wv", [D, HD], FR, kind="ExternalInput").ap()
    wo = nc.dram_tensor("wo", [HD, D], FR, kind="ExternalInput").ap()
    out = nc.dram_tensor("out", [S, D], F32, kind="ExternalOutput").ap()

    with tile.TileContext(nc) as tc:
        with ExitStack() as persist:
            const_pool = persist.enter_context(tc.tile_pool(name="const", bufs=1))
            data_pool = persist.enter_context(tc.tile_pool(name="data", bufs=1))
            w_pool = persist.enter_context(tc.tile_pool(name="wpool", bufs=1))

            ones8_f32 = const_pool.tile([P, NH], F32, tag="ones8", name="ones8_f32")
            nc.vector.memset(ones8_f32[:], 1.0)

            QT = [data_pool.tile([P, S], FR, tag=f"qt{p}", name=f"qt{p}")
                  for p in range(NPAIR)]
            KT = [data_pool.tile([P, S], FR, tag=f"kt{p}", name=f"kt{p}")
                  for p in range(NPAIR)]
            # [128 tokens, 8 heads x (64 dims + ones col)]
            VS = [data_pool.tile([P, NH * 65], FR, tag=f"vs{j}", name=f"vs{j}")
                  for j in range(NJT)]

            wq_t = w_pool.tile([P, NKT, HD], FR, tag="wq", name="wq_t")
            nc.sync.dma_start(wq_t[:], wq.rearrange("(kt p) n -> p kt n", p=P))
            wk_t = w_pool.tile([P, NKT, HD], FR, tag="wk", name="wk_t")
            nc.sync.dma_start(wk_t[:], wk.rearrange("(kt p) n -> p kt n", p=P))
            wv_t = w_pool.tile([P, NKT, HD], FR, tag="wv", name="wv_t")
            nc.sync.dma_start(wv_t[:], wv.rearrange("(kt p) n -> p kt n", p=P))
            wo_p = []
            for p in range(NPAIR):
                t = w_pool.tile([P, D], FR, tag=f"wo{p}", name=f"wo{p}")
                nc.sync.dma_start(t[:], wo[p * P:(p + 1) * P, :])
                wo_p.append(t)

            # ---------------- Phase A: projections (fused A1+A2) ----------------
            with ExitStack() as es_a:
                xm_pool = es_a.enter_context(tc.tile_pool(name="xmpool", bufs=2))
                a_ps = es_a.enter_context(
                    tc.tile_pool(name="aps", bufs=8, space="PSUM"))

                for mseg in range(NMSEG):
                    xmt = xm_pool.tile([P, NKT, MSEG], FR, tag="xm", name="xm")
                    for kt in range(NKT):
                        nc.sync.dma_start(
                            xmt[:, kt, :],
                            xT[kt * P:(kt + 1) * P,
                               mseg * MSEG:(mseg + 1) * MSEG])

                    # A1: 8 accumulators ((q|k) x 4 pairs) over 8 k-tiles
                    accs = [a_ps.tile([P, MSEG], F32, tag="acc", name="acc")
                            for _ in range(8)]
                    for kt in range(NKT):
                        for p in range(NPAIR):
                            for ti, wt in ((0, wq_t), (1, wk_t)):
                                nc.tensor.matmul(
                                    accs[p * 2 + ti][:],
                                    lhsT=wt[:, kt, p * P:(p + 1) * P],
                                    rhs=xmt[:, kt, :],
                                    start=(kt == 0), stop=(kt == NKT - 1))
                    for p in range(NPAIR):
                        nc.vector.tensor_copy(
                            QT[p][:, mseg * MSEG:(mseg + 1) * MSEG],
                            accs[p * 2][:])
                        nc.vector.tensor_copy(
                            KT[p][:, mseg * MSEG:(mseg + 1) * MSEG],
                            accs[p * 2 + 1][:])

                    # A2: V projection for the 4 token tiles of this mseg
                    vaccs = [a_ps.tile([P, HD], F32, tag="acc", name="acc")
                             for _ in range(4)]
                    for kt in range(NKT):
                        for i in range(4):
                            nc.tensor.matmul(
                                vaccs[i][:],
                                lhsT=xmt[:, kt, i * P:(i + 1) * P],
                                rhs=wv_t[:, kt, :],
                                start=(kt == 0), stop=(kt == NKT - 1))
                    for i in range(4):
                        vsv = VS[mseg * 4 + i].rearrange("p (h c) -> p h c", c=65)
                        nc.vector.tensor_copy(vsv[:, :, 0:64], vaccs[i][:])
                        nc.vector.tensor_copy(vsv[:, :, 64], ones8_f32[:])

            # ------------- Phases B + C: attention + projection -------------
            with ExitStack() as es_b:
                st_ps = es_b.enter_context(
                    tc.tile_pool(name="stps", bufs=1, space="PSUM"))
                ct_ps = es_b.enter_context(
                    tc.tile_pool(name="ctps", bufs=1, space="PSUM"))
                po_ps = es_b.enter_context(
                    tc.tile_pool(name="pops", bufs=2, space="PSUM"))
                pt_pool = es_b.enter_context(tc.tile_pool(name="ptpool", bufs=1))
                nrm_pool = es_b.enter_context(tc.tile_pool(name="nrmpool", bufs=1))
                cth_pool = es_b.enter_context(tc.tile_pool(name="cthpool", bufs=2))
                po_pool = es_b.enter_context(tc.tile_pool(name="popool", bufs=2))

                def emit_c_block(cth_prev, qb_prev, blk):
                    """One output block (128 tokens x 512 dims) of phase C."""
                    mtl, half = blk // 2, blk % 2
                    mt = qb_prev * 4 + mtl
                    po = po_ps.tile([P, 512], F32, tag="po", name="po")
                    for p in range(NPAIR):
                        nc.tensor.matmul(
                            po[:],
                            lhsT=cth_prev[p][:, mtl * P:(mtl + 1) * P],
                            rhs=wo_p[p][:, half * 512:(half + 1) * 512],
                            start=(p == 0), stop=(p == NPAIR - 1))
                    po_sb = po_pool.tile([P, 512], F32, tag="posb", name="po_sb")
                    nc.vector.tensor_copy(po_sb[:], po[:])
                    nc.sync.dma_start(
                        out[mt * P:(mt + 1) * P, half * 512:(half + 1) * 512],
                        po_sb[:])

                cth_prev = None
                qb_prev = -1
                for qb in range(NQB):
                    cth_cur = []
                    for p in range(NPAIR):
                        cts = [ct_ps.tile([65, QB], F32, tag="cte", name="cte"),
                               ct_ps.tile([65, QB], F32, tag="cto", name="cto")]
                        for jt in range(NJT):
                            par_tag = f"st{jt % 2}"
                            stg = st_ps.tile([P, 1024], F32, tag=par_tag,
                                             name="stg")
                            for par in range(2):
                                nc.tensor.matmul(
                                    stg[:, par * 512:(par + 1) * 512],
                                    lhsT=KT[p][par * 64:(par + 1) * 64,
                                               jt * P:(jt + 1) * P],
                                    rhs=QT[p][par * 64:(par + 1) * 64,
                                              qb * QB:(qb + 1) * QB],
                                    start=True, stop=True)
                            ptg = pt_pool.tile([P, 1024], FR, tag=par_tag,
                                               name="ptg")
                            nc.scalar.activation(
                                ptg[:], stg[:],
                                mybir.ActivationFunctionType.Exp, scale=0.125)
                            for par in range(2):
                                h = 2 * p + par
                                nc.tensor.matmul(
                                    cts[par][:],
                                    lhsT=VS[jt][:, h * 65:(h + 1) * 65],
                                    rhs=ptg[:, par * 512:(par + 1) * 512],
                                    start=(jt == 0), stop=(jt == NJT - 1))

                        # normalize: cth[par] = ct[0:64] * (1/ct[64]) broadcast
                        sums = nrm_pool.tile([1, 2 * QB], F32, tag="sums",
                                             name="sums")
                        nc.vector.tensor_copy(sums[:, 0:QB], cts[0][64:65, :])
                        nc.vector.tensor_copy(sums[:, QB:2 * QB],
                                              cts[1][64:65, :])
                        rcp = nrm_pool.tile([1, 2 * QB], F32, tag="rcp",
                                            name="rcp")
                        nc.vector.reciprocal_approx_fast(rcp[:], sums[:])
                        cth_t = cth_pool.tile([P, QB], FR, tag=f"cth{p}",
                                              name=f"cth{p}")
                        for par in range(2):
                            bc = nrm_pool.tile([64, QB], F32, tag=f"bc{par}",
                                               name=f"bc{par}")
                            nc.gpsimd.partition_broadcast(
                                bc[:], rcp[:, par * QB:(par + 1) * QB],
                                channels=64)
                            nc.vector.tensor_tensor(
                                cth_t[par * 64:(par + 1) * 64, :],
                                cts[par][0:64, :], bc[:],
                                mybir.AluOpType.mult)
                        cth_cur.append(cth_t)

                        # software-pipelined phase C of the previous qb
                        if cth_prev is not None:
                            emit_c_block(cth_prev, qb_prev, 2 * p)
                            emit_c_block(cth_prev, qb_prev, 2 * p + 1)

                    cth_prev, qb_prev = cth_cur, qb

                for blk in range(8):
                    emit_c_block(cth_prev, qb_prev, blk)
    nc.compile()
    return nc


_NC_CACHE = {}


def _get_nc(S=2048):
    if S not in _NC_CACHE:
        _NC_CACHE[S] = build_nc(S)
    return _NC_CACHE[S]


def kernel(x, Wq, Wk, Wv, Wo, bo):
    from concourse.bass_utils import run_bass_kernel_spmd

    x = np.asarray(x, dtype=np.float32)
    Wq = np.asarray(Wq, dtype=np.float32)
    Wk = np.asarray(Wk, dtype=np.float32)
    Wv = np.asarray(Wv, dtype=np.float32)
    Wo = np.asarray(Wo, dtype=np.float32)
    bo = np.asarray(bo, dtype=np.float32)

    bs, S, d = x.shape
    nc = _get_nc(S)

    in_maps = []
    for c in range(8):
        b, g = divmod(c, 2)
        cols = slice(g * HD, (g + 1) * HD)
        in_maps.append({
            "xT": np.ascontiguousarray(x[b].T),
            "wq": np.ascontiguousarray(Wq[:, cols]),
            "wk": np.ascontiguousarray(Wk[:, cols]),
            "wv": np.ascontiguousarray(Wv[:, cols]),
            "wo": np.ascontiguousarray(Wo[cols, :]),
        })

    res = run_bass_kernel_spmd(nc, in_maps, core_ids=list(range(8)))
    outp = np.empty((bs, S, d), dtype=np.float32)
    for b in range(bs):
        outp[b] = res.results[2 * b]["out"] + res.results[2 * b + 1]["out"] + bo
    return outp


# revision 7
# speedup vs baseline: 2.0231x; 1.1350x over previous
"""Multi-head self-attention Trainium2 kernel (8 NeuronCores).

Sharding: 8 cores = 4 batches x 2 head-groups (8 heads each).
Core c handles batch b=c//2, heads [g*8, (g+1)*8) where g=c%2.
Each core computes a partial output (its heads' contribution to the
output projection); the host sums the two partials per batch and adds bo.

All matmuls run in float32r (fp32 data, ~1 cycle/row vs 4 for fp32,
~1.5e-4 matmul rel err). fp32r matmuls require output base partition 0.

Per-core dataflow (v2 — pipelined for ACT saturation + PE density):
  xT [1024, 2048] (= x[b].T), wq/wk/wv [1024, 512], wo [512, 1024]
  A (fused): per 512-token mseg, load all 8 x k-chunks ONCE into SBUF,
     then A1: QT[p]/KT[p] = w_p.T @ x.T (8 PSUM accs over k-tiles) and
     A2: VS[jt] = [x_jt @ wv | ones] reusing the same chunks.
  B: per (pair p, 512-query block qb), per key-tile jt (16):
       2 row-packed score MMs (par0 rows 0-63, par1 rows 64-127, K=64)
         -> stg [128, 1024] (2 PSUM banks, double-buffered by jt parity)
       exp via ScalarE on the full [128, 1024] group -> ptg (SBUF, f32r)
       2 PV MMs accumulate ct[par] [65, 512] over jt (ones col => row 64
         collects sum(exp) = softmax normalizer)
     normalize: recip_approx_fast on the two sum rows, GPSIMD
       partition_broadcast, DVE mult -> cth pair tile [128, 512]
       (two heads stacked on partitions for pair-packed C).
  C: out[tokens] = sum_p cth_p.T-slice @ wo_p (K=128, 4-pair PSUM accum);
     emitted software-pipelined inside the NEXT qb's pair loop so the
     PE work hides under the ACT-bound B phase.
"""

import numpy as np

import concourse.bass as bass
import concourse.tile as tile
from concourse import bacc, mybir
from contextlib import ExitStack

P = 128
D = 1024
HD = 512  # head dims per core (8 heads x 64)
NPAIR = 4
NH = 8
F32 = mybir.dt.float32
FR = mybir.dt.float32r


def build_nc(S=2048):
    NKT = D // P          # 8 k-tiles over model dim
    NJT = S // P          # 16 key tiles
    MSEG = 512
    NMSEG = S // MSEG
    QB = 512
    NQB = S // QB

    nc = bacc.Bacc("TRN2", target_bir_lowering=False, debug=False)
    xT = nc.dram_tensor("xT", [D, S], FR, kind="ExternalInput").ap()
    wq = nc.dram_tensor("wq", [D, HD], FR, kind="ExternalInput").ap()
    wk = nc.dram_tensor("wk", [D, HD], FR, kind="ExternalInput").ap()
    wv = nc.dram_tensor("wv", [D, HD], FR, kind="ExternalInput").ap()
    wo = nc.dram_tensor("wo", [HD, D], FR, kind="ExternalInput").ap()
    out = nc.dram_tensor("out", [S, D], F32, kind="ExternalOutput").ap()

    with tile.TileContext(nc) as tc:
        with ExitStack() as persist:
            const_pool = persist.enter_context(tc.tile_pool(name="const", bufs=1))
            data_pool = persist.enter_context(tc.tile_pool(name="data", bufs=1))
            w_pool = persist.enter_context(tc.tile_pool(name="wpool", bufs=1))

            ones8_f32 = const_pool.tile([P, NH], F32, tag="ones8", name="ones8_f32")
            nc.vector.memset(ones8_f32[:], 1.0)

            QT = [data_pool.tile([P, S], FR, tag=f"qt{p}", name=f"qt{p}")
                  for p in range(NPAIR)]
            KT = [data_pool.tile([P, S], FR, tag=f"kt{p}", name=f"kt{p}")
                  for p in range(NPAIR)]
            # [128 tokens, 8 heads x (64 dims + ones col)]
            VS = [data_pool.tile([P, NH * 65], FR, tag=f"vs{j}", name=f"vs{j}")
                  for j in range(NJT)]

            # DMA order matters: wq/wk first (A1 needs them immediately),
            # then the first mseg's x chunks, then wv (A2), wo last (C only).
            wq_t = w_pool.tile([P, NKT, HD], FR, tag="wq", name="wq_t")
            nc.sync.dma_start(wq_t[:], wq.rearrange("(kt p) n -> p kt n", p=P))
            wk_t = w_pool.tile([P, NKT, HD], FR, tag="wk", name="wk_t")
            nc.sync.dma_start(wk_t[:], wk.rearrange("(kt p) n -> p kt n", p=P))
            wv_t = w_pool.tile([P, NKT, HD], FR, tag="wv", name="wv_t")
            wo_p = []

            # ---------------- Phase A: projections (fused A1+A2) ----------------
            with ExitStack() as es_a:
                xm_pool = es_a.enter_context(tc.tile_pool(name="xmpool", bufs=2))
                a_ps = es_a.enter_context(
                    tc.tile_pool(name="aps", bufs=8, space="PSUM"))

                for mseg in range(NMSEG):
                    xmc = [xm_pool.tile([P, MSEG], FR, tag=f"xm{kt}", name="xm")
                           for kt in range(NKT)]
                    for kt in range(NKT):
                        nc.sync.dma_start(
                            xmc[kt][:],
                            xT[kt * P:(kt + 1) * P,
                               mseg * MSEG:(mseg + 1) * MSEG])
                    if mseg == 0:
                        nc.sync.dma_start(
                            wv_t[:], wv.rearrange("(kt p) n -> p kt n", p=P))
                        for p in range(NPAIR):
                            t = w_pool.tile([P, D], FR, tag=f"wo{p}",
                                            name=f"wo{p}")
                            nc.sync.dma_start(t[:], wo[p * P:(p + 1) * P, :])
                            wo_p.append(t)

                    # A1: 8 accumulators ((q|k) x 4 pairs) over 8 k-tiles
                    accs = [a_ps.tile([P, MSEG], F32, tag="acc", name="acc")
                            for _ in range(8)]
                    for kt in range(NKT):
                        for p in range(NPAIR):
                            for ti, wt in ((0, wq_t), (1, wk_t)):
                                nc.tensor.matmul(
                                    accs[p * 2 + ti][:],
                                    lhsT=wt[:, kt, p * P:(p + 1) * P],
                                    rhs=xmc[kt][:],
                                    start=(kt == 0), stop=(kt == NKT - 1))
                    for p in range(NPAIR):
                        nc.vector.tensor_copy(
                            QT[p][:, mseg * MSEG:(mseg + 1) * MSEG],
                            accs[p * 2][:])
                        nc.vector.tensor_copy(
                            KT[p][:, mseg * MSEG:(mseg + 1) * MSEG],
                            accs[p * 2 + 1][:])

                    # A2: V projection for the 4 token tiles of this mseg
                    vaccs = [a_ps.tile([P, HD], F32, tag="acc", name="acc")
                             for _ in range(4)]
                    for kt in range(NKT):
                        for i in range(4):
                            nc.tensor.matmul(
                                vaccs[i][:],
                                lhsT=xmc[kt][:, i * P:(i + 1) * P],
                                rhs=wv_t[:, kt, :],
                                start=(kt == 0), stop=(kt == NKT - 1))
                    for i in range(4):
                        vsv = VS[mseg * 4 + i].rearrange("p (h c) -> p h c", c=65)
                        nc.vector.tensor_copy(vsv[:, :, 0:64], vaccs[i][:])
                        nc.vector.tensor_copy(vsv[:, :, 64], ones8_f32[:])

            # ------------- Phases B + C: attention + projection -------------
            with ExitStack() as es_b:
                st_ps = es_b.enter_context(
                    tc.tile_pool(name="stps", bufs=1, space="PSUM"))
                ct_ps = es_b.enter_context(
                    tc.tile_pool(name="ctps", bufs=1, space="PSUM"))
                po_ps = es_b.enter_context(
                    tc.tile_pool(name="pops", bufs=2, space="PSUM"))
                pt_pool = es_b.enter_context(tc.tile_pool(name="ptpool", bufs=1))
                nrm_pool = es_b.enter_context(tc.tile_pool(name="nrmpool", bufs=1))
                cth_pool = es_b.enter_context(tc.tile_pool(name="cthpool", bufs=2))
                po_pool = es_b.enter_context(tc.tile_pool(name="popool", bufs=2))

                def emit_c_block(cth_prev, qb_prev, blk):
                    """One output block (128 tokens x 512 dims) of phase C."""
                    mtl, half = blk // 2, blk % 2
                    mt = qb_prev * 4 + mtl
                    po = po_ps.tile([P, 512], F32, tag="po", name="po")
                    for p in range(NPAIR):
                        nc.tensor.matmul(
                            po[:],
                            lhsT=cth_prev[p][:, mtl * P:(mtl + 1) * P],
                            rhs=wo_p[p][:, half * 512:(half + 1) * 512],
                            start=(p == 0), stop=(p == NPAIR - 1))
                    po_sb = po_pool.tile([P, 512], F32, tag="posb", name="po_sb")
                    nc.vector.tensor_copy(po_sb[:], po[:])
                    nc.sync.dma_start(
                        out[mt * P:(mt + 1) * P, half * 512:(half + 1) * 512],
                        po_sb[:])

                cth_prev = None
                qb_prev = -1
                for qb in range(NQB):
                    cth_cur = []
                    for p in range(NPAIR):
                        cts = [ct_ps.tile([65, QB], F32, tag="cte", name="cte"),
                               ct_ps.tile([65, QB], F32, tag="cto", name="cto")]
                        for jt in range(NJT):
                            par_tag = f"st{jt % 2}"
                            stg = st_ps.tile([P, 1024], F32, tag=par_tag,
                                             name="stg")
                            for par in range(2):
                                nc.tensor.matmul(
                                    stg[:, par * 512:(par + 1) * 512],
                                    lhsT=KT[p][par * 64:(par + 1) * 64,
                                               jt * P:(jt + 1) * P],
                                    rhs=QT[p][par * 64:(par + 1) * 64,
                                              qb * QB:(qb + 1) * QB],
                                    start=True, stop=True)
                            ptg = pt_pool.tile([P, 1024], FR, tag=par_tag,
                                               name="ptg")
                            nc.scalar.activation(
                                ptg[:], stg[:],
                                mybir.ActivationFunctionType.Exp, scale=0.125)
                            for par in range(2):
                                h = 2 * p + par
                                nc.tensor.matmul(
                                    cts[par][:],
                                    lhsT=VS[jt][:, h * 65:(h + 1) * 65],
                                    rhs=ptg[:, par * 512:(par + 1) * 512],
                                    start=(jt == 0), stop=(jt == NJT - 1))
                            # phase C of the previous qb, spread mid-pair so
                            # the PE work hides under the ACT-bound pipeline
                            if cth_prev is not None:
                                if jt == 6:
                                    emit_c_block(cth_prev, qb_prev, 2 * p)
                                elif jt == 14:
                                    emit_c_block(cth_prev, qb_prev, 2 * p + 1)

                        # Copy ct out of PSUM right away (releases the banks
                        # for the next pair); normalize entirely on SBUF —
                        # cth isn't consumed until the next qb's C blocks.
                        ctu = [nrm_pool.tile([65, QB], F32, tag=f"ctu{par}",
                                             name=f"ctu{par}")
                               for par in range(2)]
                        for par in range(2):
                            nc.vector.tensor_copy(ctu[par][:], cts[par][:])
                        sums = nrm_pool.tile([1, 2 * QB], F32, tag="sums",
                                             name="sums")
                        nc.vector.tensor_copy(sums[:, 0:QB], ctu[0][64:65, :])
                        nc.vector.tensor_copy(sums[:, QB:2 * QB],
                                              ctu[1][64:65, :])
                        rcp = nrm_pool.tile([1, 2 * QB], F32, tag="rcp",
                                            name="rcp")
                        nc.vector.reciprocal_approx_fast(rcp[:], sums[:])
                        cth_t = cth_pool.tile([P, QB], FR, tag=f"cth{p}",
                                              name=f"cth{p}")
                        for par in range(2):
                            bc = nrm_pool.tile([64, QB], F32, tag=f"bc{par}",
                                               name=f"bc{par}")
                            nc.gpsimd.partition_broadcast(
                                bc[:], rcp[:, par * QB:(par + 1) * QB],
                                channels=64)
                            nc.vector.tensor_tensor(
                                cth_t[par * 64:(par + 1) * 64, :],
                                ctu[par][0:64, :], bc[:],
                                mybir.AluOpType.mult)
                        cth_cur.append(cth_t)

                    cth_prev, qb_prev = cth_cur, qb

                for blk in range(8):
                    emit_c_block(cth_prev, qb_prev, blk)
    nc.compile()
    return nc


_NC_CACHE = {}


def _get_nc(S=2048):
    if S not in _NC_CACHE:
        _NC_CACHE[S] = build_nc(S)
    return _NC_CACHE[S]


def kernel(x, Wq, Wk, Wv, Wo, bo):
    from concourse.bass_utils import run_bass_kernel_spmd

    x = np.asarray(x, dtype=np.float32)
    Wq = np.asarray(Wq, dtype=np.float32)
    Wk = np.asarray(Wk, dtype=np.float32)
    Wv = np.asarray(Wv, dtype=np.float32)
    Wo = np.asarray(Wo, dtype=np.float32)
    bo = np.asarray(bo, dtype=np.float32)

    bs, S, d = x.shape
    nc = _get_nc(S)

    in_maps = []
    for c in range(8):
        b, g = divmod(c, 2)
        cols = slice(g * HD, (g + 1) * HD)
        in_maps.append({
            "xT": np.ascontiguousarray(x[b].T),
            "wq": np.ascontiguousarray(Wq[:, cols]),
            "wk": np.ascontiguousarray(Wk[:, cols]),
            "wv": np.ascontiguousarray(Wv[:, cols]),
            "wo": np.ascontiguousarray(Wo[cols, :]),
        })

    res = run_bass_kernel_spmd(nc, in_maps, core_ids=list(range(8)))
    outp = np.empty((bs, S, d), dtype=np.float32)
    for b in range(bs):
        outp[b] = res.results[2 * b]["out"] + res.results[2 * b + 1]["out"] + bo
    return outp


# revision 9
# speedup vs baseline: 2.0608x; 1.0186x over previous
"""Multi-head self-attention Trainium2 kernel (8 NeuronCores).

Sharding: 8 cores = 4 batches x 2 head-groups (8 heads each).
Core c handles batch b=c//2, heads [g*8, (g+1)*8) where g=c%2.
Each core computes a partial output (its heads' contribution to the
output projection); the host sums the two partials per batch and adds bo.

All matmuls run in float32r (fp32 data, ~1 cycle/row vs 4 for fp32,
~1.5e-4 matmul rel err). fp32r matmuls require output base partition 0.

Per-core dataflow (v2 — pipelined for ACT saturation + PE density):
  xT [1024, 2048] (= x[b].T), wq/wk/wv [1024, 512], wo [512, 1024]
  A (fused): per 512-token mseg, load all 8 x k-chunks ONCE into SBUF,
     then A1: QT[p]/KT[p] = w_p.T @ x.T (8 PSUM accs over k-tiles) and
     A2: VS[jt] = [x_jt @ wv | ones] reusing the same chunks.
  B: per (pair p, 512-query block qb), per key-tile jt (16):
       2 row-packed score MMs (par0 rows 0-63, par1 rows 64-127, K=64)
         -> stg [128, 1024] (2 PSUM banks, double-buffered by jt parity)
       exp via ScalarE on the full [128, 1024] group -> ptg (SBUF, f32r)
       2 PV MMs accumulate ct[par] [65, 512] over jt (ones col => row 64
         collects sum(exp) = softmax normalizer)
     normalize: recip_approx_fast on the two sum rows, GPSIMD
       partition_broadcast, DVE mult -> cth pair tile [128, 512]
       (two heads stacked on partitions for pair-packed C).
  C: out[tokens] = sum_p cth_p.T-slice @ wo_p (K=128, 4-pair PSUM accum);
     emitted software-pipelined inside the NEXT qb's pair loop so the
     PE work hides under the ACT-bound B phase.
"""

import numpy as np

import concourse.bass as bass
import concourse.tile as tile
from concourse import bacc, mybir
from contextlib import ExitStack

P = 128
D = 1024
HD = 512  # head dims per core (8 heads x 64)
NPAIR = 4
NH = 8
F32 = mybir.dt.float32
FR = mybir.dt.float32r


def build_nc(S=2048):
    NKT = D // P          # 8 k-tiles over model dim
    NJT = S // P          # 16 key tiles
    MSEG = 512
    NMSEG = S // MSEG
    QB = 512
    NQB = S // QB

    nc = bacc.Bacc("TRN2", target_bir_lowering=False, debug=False)
    xT = nc.dram_tensor("xT", [D, S], FR, kind="ExternalInput").ap()
    wq = nc.dram_tensor("wq", [D, HD], FR, kind="ExternalInput").ap()
    wk = nc.dram_tensor("wk", [D, HD], FR, kind="ExternalInput").ap()
    wv = nc.dram_tensor("wv", [D, HD], FR, kind="ExternalInput").ap()
    wo = nc.dram_tensor("wo", [HD, D], FR, kind="ExternalInput").ap()
    out = nc.dram_tensor("out", [S, D], F32, kind="ExternalOutput").ap()

    with tile.TileContext(nc) as tc:
        with ExitStack() as persist:
            const_pool = persist.enter_context(tc.tile_pool(name="const", bufs=1))
            data_pool = persist.enter_context(tc.tile_pool(name="data", bufs=1))
            w_pool = persist.enter_context(tc.tile_pool(name="wpool", bufs=1))

            ones8_f32 = const_pool.tile([P, NH], F32, tag="ones8", name="ones8_f32")
            nc.vector.memset(ones8_f32[:], 1.0)

            QT = [data_pool.tile([P, S], FR, tag=f"qt{p}", name=f"qt{p}")
                  for p in range(NPAIR)]
            KT = [data_pool.tile([P, S], FR, tag=f"kt{p}", name=f"kt{p}")
                  for p in range(NPAIR)]
            # [128 tokens, 8 heads x (64 dims + ones col)]
            VS = [data_pool.tile([P, NH * 65], FR, tag=f"vs{j}", name=f"vs{j}")
                  for j in range(NJT)]

            # Weights stream on the Activation DMA queue (idle during phase A)
            # while x chunks stream on the sync queue, per k-tile so the first
            # matmul only waits for the first chunks of each.
            wq_c = [w_pool.tile([P, HD], FR, tag=f"wq{kt}", name=f"wq{kt}")
                    for kt in range(NKT)]
            wk_c = [w_pool.tile([P, HD], FR, tag=f"wk{kt}", name=f"wk{kt}")
                    for kt in range(NKT)]
            wv_c = [w_pool.tile([P, HD], FR, tag=f"wv{kt}", name=f"wv{kt}")
                    for kt in range(NKT)]
            for kt in range(NKT):
                nc.scalar.dma_start(wq_c[kt][:], wq[kt * P:(kt + 1) * P, :])
                nc.scalar.dma_start(wk_c[kt][:], wk[kt * P:(kt + 1) * P, :])
            for kt in range(NKT):
                nc.scalar.dma_start(wv_c[kt][:], wv[kt * P:(kt + 1) * P, :])
            wo_p = []
            for p in range(NPAIR):
                t = w_pool.tile([P, D], FR, tag=f"wo{p}", name=f"wo{p}")
                nc.scalar.dma_start(t[:], wo[p * P:(p + 1) * P, :])
                wo_p.append(t)

            # warm up GPSIMD (first instruction on it costs ~8us)
            gp_warm = const_pool.tile([64, 8], F32, tag="gpw", name="gp_warm")
            nc.gpsimd.partition_broadcast(gp_warm[:], ones8_f32[0:1, 0:8],
                                          channels=64)

            # ---------------- Phase A: projections (fused A1+A2) ----------------
            with ExitStack() as es_a:
                xm_pool = es_a.enter_context(tc.tile_pool(name="xmpool", bufs=2))
                a_ps = es_a.enter_context(
                    tc.tile_pool(name="aps", bufs=8, space="PSUM"))

                for mseg in range(NMSEG):
                    xmc = [xm_pool.tile([P, MSEG], FR, tag=f"xm{kt}", name="xm")
                           for kt in range(NKT)]
                    for kt in range(NKT):
                        nc.sync.dma_start(
                            xmc[kt][:],
                            xT[kt * P:(kt + 1) * P,
                               mseg * MSEG:(mseg + 1) * MSEG])

                    # A1: 8 accumulators ((q|k) x 4 pairs) over 8 k-tiles
                    accs = [a_ps.tile([P, MSEG], F32, tag="acc", name="acc")
                            for _ in range(8)]
                    for kt in range(NKT):
                        for p in range(NPAIR):
                            for ti, wt in ((0, wq_c), (1, wk_c)):
                                nc.tensor.matmul(
                                    accs[p * 2 + ti][:],
                                    lhsT=wt[kt][:, p * P:(p + 1) * P],
                                    rhs=xmc[kt][:],
                                    start=(kt == 0), stop=(kt == NKT - 1))
                    for p in range(NPAIR):
                        nc.vector.tensor_copy(
                            QT[p][:, mseg * MSEG:(mseg + 1) * MSEG],
                            accs[p * 2][:])
                        nc.vector.tensor_copy(
                            KT[p][:, mseg * MSEG:(mseg + 1) * MSEG],
                            accs[p * 2 + 1][:])

                    # A2: V projection for the 4 token tiles of this mseg
                    vaccs = [a_ps.tile([P, HD], F32, tag="acc", name="acc")
                             for _ in range(4)]
                    for kt in range(NKT):
                        for i in range(4):
                            nc.tensor.matmul(
                                vaccs[i][:],
                                lhsT=xmc[kt][:, i * P:(i + 1) * P],
                                rhs=wv_c[kt][:],
                                start=(kt == 0), stop=(kt == NKT - 1))
                    for i in range(4):
                        vsv = VS[mseg * 4 + i].rearrange("p (h c) -> p h c", c=65)
                        nc.vector.tensor_copy(vsv[:, :, 0:64], vaccs[i][:])
                        nc.vector.tensor_copy(vsv[:, :, 64], ones8_f32[:])

            # ------------- Phases B + C: attention + projection -------------
            with ExitStack() as es_b:
                st_ps = es_b.enter_context(
                    tc.tile_pool(name="stps", bufs=1, space="PSUM"))
                ct_ps = es_b.enter_context(
                    tc.tile_pool(name="ctps", bufs=1, space="PSUM"))
                po_ps = es_b.enter_context(
                    tc.tile_pool(name="pops", bufs=2, space="PSUM"))
                pt_pool = es_b.enter_context(tc.tile_pool(name="ptpool", bufs=1))
                nrm_pool = es_b.enter_context(tc.tile_pool(name="nrmpool", bufs=1))
                cth_pool = es_b.enter_context(tc.tile_pool(name="cthpool", bufs=2))
                po_pool = es_b.enter_context(tc.tile_pool(name="popool", bufs=2))

                def emit_c_block(cth_prev, qb_prev, blk):
                    """One output block (128 tokens x 512 dims) of phase C."""
                    mtl, half = blk // 2, blk % 2
                    mt = qb_prev * 4 + mtl
                    po = po_ps.tile([P, 512], F32, tag="po", name="po")
                    for p in range(NPAIR):
                        nc.tensor.matmul(
                            po[:],
                            lhsT=cth_prev[p][:, mtl * P:(mtl + 1) * P],
                            rhs=wo_p[p][:, half * 512:(half + 1) * 512],
                            start=(p == 0), stop=(p == NPAIR - 1))
                    po_sb = po_pool.tile([P, 512], F32, tag="posb", name="po_sb")
                    nc.vector.tensor_copy(po_sb[:], po[:])
                    nc.sync.dma_start(
                        out[mt * P:(mt + 1) * P, half * 512:(half + 1) * 512],
                        po_sb[:])

                # Flattened (qb, pair, jt) group loop with the score MMs
                # emitted one group ahead, so exp never waits on a fresh
                # semaphore chain (ACT is the pacing engine in phase B).
                groups = [(qb, p, jt)
                          for qb in range(NQB)
                          for p in range(NPAIR)
                          for jt in range(NJT)]

                def emit_s(g):
                    qb, p, jt = groups[g]
                    stg = st_ps.tile([P, 1024], F32, tag=f"st{g % 2}",
                                     name="stg")
                    for par in range(2):
                        nc.tensor.matmul(
                            stg[:, par * 512:(par + 1) * 512],
                            lhsT=KT[p][par * 64:(par + 1) * 64,
                                       jt * P:(jt + 1) * P],
                            rhs=QT[p][par * 64:(par + 1) * 64,
                                      qb * QB:(qb + 1) * QB],
                            start=True, stop=True)
                    return stg

                cth_prev = None
                qb_prev = -1
                cth_cur = []
                cts = None
                stg_next = emit_s(0)
                for g, (qb, p, jt) in enumerate(groups):
                    stg = stg_next
                    if jt == 0:
                        cts = [ct_ps.tile([65, QB], F32, tag="cte", name="cte"),
                               ct_ps.tile([65, QB], F32, tag="cto", name="cto")]
                    if g + 1 < len(groups):
                        stg_next = emit_s(g + 1)
                    ptg = pt_pool.tile([P, 1024], FR, tag=f"pt{g % 2}",
                                       name="ptg")
                    nc.scalar.activation(
                        ptg[:], stg[:],
                        mybir.ActivationFunctionType.Exp, scale=0.125)
                    for par in range(2):
                        h = 2 * p + par
                        nc.tensor.matmul(
                            cts[par][:],
                            lhsT=VS[jt][:, h * 65:(h + 1) * 65],
                            rhs=ptg[:, par * 512:(par + 1) * 512],
                            start=(jt == 0), stop=(jt == NJT - 1))
                    # phase C of the previous qb, spread mid-pair so the PE
                    # work hides under the ACT-bound pipeline
                    if cth_prev is not None:
                        if jt == 6:
                            emit_c_block(cth_prev, qb_prev, 2 * p)
                        elif jt == 14:
                            emit_c_block(cth_prev, qb_prev, 2 * p + 1)

                    if jt == NJT - 1:
                        # Copy ct out of PSUM right away (releases the banks
                        # for the next pair); normalize entirely on SBUF —
                        # cth isn't consumed until the next qb's C blocks.
                        ctu = [nrm_pool.tile([65, QB], F32, tag=f"ctu{par}",
                                             name=f"ctu{par}")
                               for par in range(2)]
                        for par in range(2):
                            nc.vector.tensor_copy(ctu[par][:], cts[par][:])
                        sums = nrm_pool.tile([1, 2 * QB], F32, tag="sums",
                                             name="sums")
                        nc.vector.tensor_copy(sums[:, 0:QB], ctu[0][64:65, :])
                        nc.vector.tensor_copy(sums[:, QB:2 * QB],
                                              ctu[1][64:65, :])
                        rcp = nrm_pool.tile([1, 2 * QB], F32, tag="rcp",
                                            name="rcp")
                        nc.vector.reciprocal_approx_fast(rcp[:], sums[:])
                        cth_t = cth_pool.tile([P, QB], FR, tag=f"cth{p}",
                                              name=f"cth{p}")
                        for par in range(2):
                            bc = nrm_pool.tile([64, QB], F32, tag=f"bc{par}",
                                               name=f"bc{par}")
                            nc.gpsimd.partition_broadcast(
                                bc[:], rcp[:, par * QB:(par + 1) * QB],
                                channels=64)
                            nc.vector.tensor_tensor(
                                cth_t[par * 64:(par + 1) * 64, :],
                                ctu[par][0:64, :], bc[:],
                                mybir.AluOpType.mult)
                        cth_cur.append(cth_t)
                        if p == NPAIR - 1:
                            cth_prev, qb_prev = cth_cur, qb
                            cth_cur = []

                for blk in range(8):
                    emit_c_block(cth_prev, qb_prev, blk)
    nc.compile()
    return nc


_NC_CACHE = {}


def _get_nc(S=2048):
    if S not in _NC_CACHE:
        _NC_CACHE[S] = build_nc(S)
    return _NC_CACHE[S]


def kernel(x, Wq, Wk, Wv, Wo, bo):
    from concourse.bass_utils import run_bass_kernel_spmd

    x = np.asarray(x, dtype=np.float32)
    Wq = np.asarray(Wq, dtype=np.float32)
    Wk = np.asarray(Wk, dtype=np.float32)
    Wv = np.asarray(Wv, dtype=np.float32)
    Wo = np.asarray(Wo, dtype=np.float32)
    bo = np.asarray(bo, dtype=np.float32)

    bs, S, d = x.shape
    nc = _get_nc(S)

    in_maps = []
    for c in range(8):
        b, g = divmod(c, 2)
        cols = slice(g * HD, (g + 1) * HD)
        in_maps.append({
            "xT": np.ascontiguousarray(x[b].T),
            "wq": np.ascontiguousarray(Wq[:, cols]),
            "wk": np.ascontiguousarray(Wk[:, cols]),
            "wv": np.ascontiguousarray(Wv[:, cols]),
            "wo": np.ascontiguousarray(Wo[cols, :]),
        })

    res = run_bass_kernel_spmd(nc, in_maps, core_ids=list(range(8)))
    outp = np.empty((bs, S, d), dtype=np.float32)
    for b in range(bs):
        outp[b] = res.results[2 * b]["out"] + res.results[2 * b + 1]["out"] + bo
    return outp
